# revision 22
# baseline (speedup 1.0000x reference)
"""GAT-style attentive layer on 8 TRN2 NeuronCores — fp8 DoubleRow version.

Math (per reference):
    Wh  = input                      [N, D]   (N=8192, D=512)
    Wh1 = Wh @ a[:D]  (s_i)          [N, 1]
    Wh2 = Wh @ a[D:]  (t_j)          [N, 1]
    e   = leaky_relu(Wh1 + Wh2.T, 0.01)
    e   = where(adj > 0, e, -9e15)
    att = softmax(e, axis=1)
    out = att @ Wh                   [N, D]

Sharding: row-shard the N x N attention across 8 cores (1024 rows each).
Scores are produced transposed, pT[j, i] (j on partitions), so the final
matmul uses pT tiles as the stationary operand.

Key speed tricks (all validated against the TimelineSim cost model):
 - The big matmul runs in fp8e4 (e4m3) with MatmulPerfMode.DoubleRow: one
   PE instruction contracts TWO 128-deep j-tiles at 0.5 cycles per output
   column (4x the bf16 rate per MAC).  Accuracy is held by splitting Wh
   into hi+lo e4m3 planes (w = w_hi + w_lo reconstructs bf16-level w), so
   only the attention weights p carry fp8 noise; measured end-to-end
   rel-err 1.66e-2 against the f32 oracle (gate: 2e-2).
 - p8 values are produced by a single Act pass per score tile: the
   compiler's Exp table is patched to compute exp(lrelu(x) - 1.25), with
   x <= -20 mapped to exact 0.  The -1.25 shift keeps p below e4m3's 240
   max (softmax cancels the shift), lrelu is fused into the table, and
   the adjacency mask is additive ({0, -112} fp8), folded in by the
   score-add.
 - The score-add s_t = (adjT8 + t_j) + bcast_wh1 is a single
   scalar_tensor_tensor op (per-partition t_j rides the scalar slot),
   split between the DVE and gpsimd engines to keep both under the Act
   engine's throughput.  Folding t_j here (instead of the Act bias) lets
   the Act pass batch 4 j-tiles per instruction.
 - Row sums ride a third DoubleRow matmul per pair against a ones pair
   (~1 cycle each); normalization is a reciprocal-multiply on the DVE.

Host-side prep (data marshaling only): dtype casts + transpose/slicing;
all compute (projections, scores, exp, matmul, normalize) runs on device.
"""

import os

import numpy as np
import ml_dtypes

import concourse.bass as bass
import concourse.mybir as mybir
import concourse.tile as tile
from concourse import bacc
from concourse.bass_utils import run_bass_kernel_spmd

N = 8192          # nodes
D = 512           # feature dim
NCORES = 8
ROWS = N // NCORES  # 1024 output rows per core
P = 128
NJT = N // P      # 64 j-tiles
NPAIR = NJT // 2  # 32 j-tile pairs (one DoubleRow contraction each)
IC_W = 512        # i-chunk width (PSUM-limited)
NIC = ROWS // IC_W  # 2 i-chunks
ITPC = IC_W // P  # 4 i-subtiles per chunk

MASK_NEG = -112.0  # additive mask (e4m3-exact); table maps x <= -20 to 0
C_SHIFT = 1.25     # table computes exp(lrelu(x) - C_SHIFT): keeps p < 240

AF = mybir.ActivationFunctionType
ALU = mybir.AluOpType
PM = mybir.MatmulPerfMode
dt = mybir.dt
F32 = dt.float32
BF16 = dt.bfloat16
FP8 = dt.float8e4

# All score-adds run on the DVE: the per-partition t_j scalar rides the
# TensorScalarPtr opcode, which the Pool engine does not implement.
def _add_on_pool(jt: int, ic: int) -> bool:
    return False


def _build_kernel(nc: bass.Bass, tc: tile.TileContext,
                  adjT8: bass.AP, whi: bass.AP, wlo: bass.AP,
                  xT: bass.AP, a8t_d: bass.AP, out: bass.AP,
                  w1scr: bass.AP, ctx):
    pool_const = ctx.enter_context(tc.tile_pool(name="const", bufs=1))
    pool_w = ctx.enter_context(tc.tile_pool(name="w", bufs=1))
    pool_adj = ctx.enter_context(tc.tile_pool(name="adj", bufs=8))
    pool_st = ctx.enter_context(tc.tile_pool(name="st", bufs=3))
    pool_p8 = ctx.enter_context(tc.tile_pool(name="p8", bufs=3))
    pool_outs = ctx.enter_context(tc.tile_pool(name="outs", bufs=2))
    pool_small = ctx.enter_context(tc.tile_pool(name="small", bufs=1))
    pool_psum = ctx.enter_context(tc.tile_pool(name="psum", bufs=1, space="PSUM"))

    # ---- constants / warmup ------------------------------------------------
    warm = pool_const.tile([1, 2], F32, tag="warm", name="warm")
    nc.vector.memset(warm, 0.0)
    nc.scalar.activation(warm, warm, AF.Exp)  # pull ACT_TABLE_LOAD to t~0

    ones2 = pool_const.tile([P, 2, 2], FP8, tag="ones2", name="ones2")
    nc.vector.memset(ones2, 1.0)

    a8t = pool_const.tile([P, 8], BF16, tag="a8t", name="a8t")
    nc.sync.dma_start(a8t, a8t_d)

    # ---- PSUM arena (bank 8): t-columns [*,0:64], row-sums [*,64:80] -------
    # (psum_out takes 7 banks, pass-B 1 more would be 9: the small stuff
    # shares one bank, memset once, all matmuls start=False.)
    arena = pool_psum.tile([P, 512], F32, tag="arena", name="arena")
    nc.vector.memset(arena, 0.0)

    # ---- xT (transposed x, bf16) for wh1 + wh2 projections -----------------
    # Streamed in 8 j-range chunks of [128, 4, 1024] (1MB) through a 3-buf
    # ring, so t-projections for early j-tiles unblock fast and the full
    # 8MB never sits in SBUF.  The host rotates j per core so chunk 0 is
    # always this core's own rows (which also serve wh1).
    pool_xT = ctx.enter_context(tc.tile_pool(name="xT", bufs=2))
    xTc = {}

    def dma_xT_chunk(g):
        t = pool_xT.tile([P, 4, ROWS], BF16, tag="xTc", name="xTc", bufs=2)
        nc.sync.dma_start(
            t, xT[:, bass.ds(g * ROWS, ROWS)].rearrange("(t p) j -> p t j", p=P))
        xTc[g] = t

    dma_xT_chunk(0)

    bcast16 = pool_const.tile([P, ROWS], BF16, tag="bw", name="bw")
    wh1_row = pool_const.tile([1, ROWS], BF16, tag="w1r", name="w1r")
    wh1c_sb = pool_const.tile([P, 8], BF16, tag="w1c", name="w1c")

    def wh1_compute():
        # column-layout a1-projections of the local rows into arena[*,80:88]
        # (same machinery as t_compute), then a tiny SBUF->SBUF transpose DMA
        # to the [1, 1024] row the partition-broadcast wants.  No temporal
        # arena sharing, so nothing serializes behind re-zeros.
        for k in range(8):
            for t in range(4):
                nc.tensor.matmul(arena[:, 80 + k:81 + k],
                                 lhsT=xTc[0][:, t, bass.ds(k * P, P)],
                                 rhs=a8t[:, t:t + 1],
                                 start=False, stop=(t == 3),
                                 skip_group_check=True)
        nc.scalar.copy(wh1c_sb, arena[:, 80:88])
        nc.vector.memset(arena[:, 80:88], 0.0)  # pass-B reuses these columns
        # transpose [128, 8] -> [1, 1024] via a DRAM hop (DRAM APs are
        # free-form; a cross-partition SBUF->SBUF AP is not expressible)
        nc.sync.dma_start(
            w1scr.rearrange("x (t p) -> p (x t)", p=P), wh1c_sb)
        nc.sync.dma_start(wh1_row, w1scr)
        nc.gpsimd.partition_broadcast(bcast16, wh1_row[0:1, :])

    # t_j for ALL j (wh2), in bias-column layout [128, 64]: per j-tile a
    # [128, 1] arena column from 4 k-matmuls (ap_size=1: nearly free on PE).
    wh2_sb = pool_const.tile([P, NJT], F32, tag="wh2", name="wh2")

    def t_compute(g):
        # group g covers j-tiles 8g..8g+7 (chunk g's 1024 columns)
        for k in range(8):
            jt = 8 * g + k
            for t in range(4):
                nc.tensor.matmul(arena[:, jt:jt + 1],
                                 lhsT=xTc[g][:, t, bass.ds(k * P, P)],
                                 rhs=a8t[:, 4 + t:5 + t],
                                 start=False, stop=(t == 3),
                                 skip_group_check=True)
        nc.scalar.copy(wh2_sb[:, bass.ds(8 * g, 8)], arena[:, bass.ds(8 * g, 8)])
        del xTc[g]

    adj_pre = {}

    def dma_adj_pair(pair):
        t = pool_adj.tile([P, 2, ROWS], FP8, tag="adjq", name="adjq", bufs=10)
        nc.sync.dma_start(
            t, adjT8[bass.ds(pair * 2 * P, 2 * P), :]
            .rearrange("(q p) i -> p q i", p=P))
        return t

    adj_pre[0] = dma_adj_pair(0)
    wh1_compute()
    t_compute(0)
    adj_pre[1] = dma_adj_pair(1)
    dma_xT_chunk(1)

    # ---- W planes (resident), interleaved with adj pairs + xT chunks in
    # strict consumption order (the DMA queue is FIFO; order here IS the
    # delivery schedule).
    whq_hi, whq_lo = [], []
    for m in range(16):
        if 2 * m not in adj_pre:
            adj_pre[2 * m] = dma_adj_pair(2 * m)
        thi = pool_w.tile([P, 4, D], FP8, tag=f"whi{m}", name=f"whi{m}")
        nc.sync.dma_start(thi, whi[bass.ds(m * 4 * P, 4 * P), :]
                          .rearrange("(q p) d -> p q d", p=P))
        whq_hi.append(thi)
        if 2 * m + 1 not in adj_pre:
            adj_pre[2 * m + 1] = dma_adj_pair(2 * m + 1)
        tlo = pool_w.tile([P, 4, D], FP8, tag=f"wlo{m}", name=f"wlo{m}")
        nc.sync.dma_start(tlo, wlo[bass.ds(m * 4 * P, 4 * P), :]
                          .rearrange("(q p) d -> p q d", p=P))
        whq_lo.append(tlo)
        if m <= 5:
            dma_xT_chunk(m + 2)
        if 1 <= m <= 7:
            t_compute(m)

    # ---- single pass over j: i-subtiles 0..6 stream through 7 PSUM banks;
    # p8 tiles stay resident so i-subtile 7 runs as a short PE-only pass B.
    NA = 7  # pass-A i-subtiles
    psum_out = [pool_psum.tile([P, D], F32, tag=f"po{i}", name=f"po{i}")
                for i in range(NA)]
    p8_tiles = []

    for quad in range(16):  # 4 j-tiles per quad
        s_q = pool_st.tile([P, 4, ROWS], BF16, tag="s_q", name="s_q", bufs=2)
        for k in range(4):
            jt = 4 * quad + k
            pair = jt // 2
            if pair in adj_pre:
                adjq = adj_pre.pop(pair)
            nc.vector.scalar_tensor_tensor(
                out=s_q[:, k, :], in0=adjq[:, jt % 2, :],
                scalar=wh2_sb[:, jt:jt + 1], in1=bcast16,
                op0=ALU.add, op1=ALU.add)
        p8 = pool_p8.tile([P, 4, ROWS], FP8, tag=f"p8q{quad}",
                          name=f"p8q{quad}", bufs=1)
        p8_tiles.append(p8)
        nc.scalar.activation(p8, s_q, AF.Exp)

        for pp in range(2):
            pair = 2 * quad + pp
            first = pair == 0
            last = pair == NPAIR - 1
            # pass-B row-sum (i-subtile 7) accumulates during pass A
            lhs7 = p8[:, bass.ds(2 * pp, 2), bass.ds(NA * P, P)]
            nc.tensor.matmul(arena[:, 78:80], lhsT=lhs7, rhs=ones2,
                             start=False, stop=(pair == NPAIR - 1),
                             perf_mode=PM.DoubleRow, skip_group_check=True)
            for i4 in range(NA):
                lhs = p8[:, bass.ds(2 * pp, 2), bass.ds(i4 * P, P)]
                nc.tensor.matmul(psum_out[i4], lhsT=lhs,
                                 rhs=whq_hi[pair // 2][:, bass.ds(2 * (pair % 2), 2), :],
                                 start=first, stop=False,
                                 perf_mode=PM.DoubleRow,
                                 skip_group_check=not first)
                nc.tensor.matmul(psum_out[i4], lhsT=lhs,
                                 rhs=whq_lo[pair // 2][:, bass.ds(2 * (pair % 2), 2), :],
                                 start=False, stop=last,
                                 perf_mode=PM.DoubleRow,
                                 skip_group_check=True)
                nc.tensor.matmul(arena[:, 64 + 2 * i4:64 + 2 * i4 + 2],
                                 lhsT=lhs, rhs=ones2,
                                 start=False, stop=last,
                                 perf_mode=PM.DoubleRow,
                                 skip_group_check=True)

    # ---- pass B: i-subtile 7 from the resident p8 tiles, as two 256-wide
    # half-sweeps through free arena columns [80:336] (no dependency on the
    # pass-A reciprocal / bank evacuation).
    recip16 = pool_small.tile([P, 16], F32, tag="recip", name="recip")
    nc.vector.reciprocal(recip16[:, 14:16], arena[:, 78:80])
    outb = pool_outs.tile([P, 2, 256], BF16, tag="outb", name="outb")
    for half in range(2):
        pb = arena[:, 80:336]
        for pair in range(NPAIR):
            p8 = p8_tiles[pair // 2]
            lhs = p8[:, bass.ds(2 * (pair % 2), 2), bass.ds(NA * P, P)]
            nc.tensor.matmul(
                pb, lhsT=lhs,
                rhs=whq_hi[pair // 2][:, bass.ds(2 * (pair % 2), 2),
                                      bass.ds(half * 256, 256)],
                start=False, stop=False,
                perf_mode=PM.DoubleRow, skip_group_check=True)
            nc.tensor.matmul(
                pb, lhsT=lhs,
                rhs=whq_lo[pair // 2][:, bass.ds(2 * (pair % 2), 2),
                                      bass.ds(half * 256, 256)],
                start=False, stop=(pair == NPAIR - 1),
                perf_mode=PM.DoubleRow, skip_group_check=True)
        nc.vector.tensor_scalar_mul(outb[:, half, :], pb, recip16[:, 14:15])
        if half == 0:
            nc.vector.memset(pb, 0.0)
    nc.sync.dma_start(out[bass.ds(NA * P, P), :], outb)



    # ---- pass A normalize + ship (i-subtiles 0..6) -------------------------
    nc.vector.reciprocal(recip16[:, 0:14], arena[:, 64:78])
    outq = pool_outs.tile([P, NA, D], BF16, tag="outq", name="outq")
    for i4 in range(NA):
        recip = recip16[:, 2 * i4:2 * i4 + 1]
        if i4 % 2 == 0:
            nc.vector.tensor_scalar_mul(outq[:, i4, :], psum_out[i4], recip)
        else:
            nc.scalar.mul(outq[:, i4, :], psum_out[i4], recip)
        if i4 % 2 == 1:
            nc.sync.dma_start(
                out[bass.ds((i4 - 1) * P, 2 * P), :].rearrange(
                    "(q p) d -> p q d", p=P), outq[:, i4 - 1:i4 + 1, :])
    nc.sync.dma_start(out[bass.ds(6 * P, P), :], outq[:, 6, :])

_CACHED = None

_FUSED_ALPHA = 0.01
_ZERO_BELOW = -20.0  # table inputs below this produce exact 0


def _make_fused_act_root() -> str:
    """Copy the compiler's activation-table dir, patching Exp to compute
      x < -20:       exactly 0 (additively-masked scores exp to zero)
      x in [-20, 0): exp(_FUSED_ALPHA*x - C_SHIFT)   (lrelu fused)
      x >= 0:        exp(x - C_SHIFT)
    The -C_SHIFT keeps outputs under e4m3's 240 max; softmax cancels it.
    Returns path to the patched act_info.json."""
    import json
    import shutil
    import tempfile

    from neuronxcc.driver.Job import Job
    from neuronxcc.driver.jobs.support.FindActInfo import findActInfoFile

    src_root = os.path.dirname(findActInfoFile(Job.getPackageDir(), "gen3"))
    dst = tempfile.mkdtemp(prefix="act_root_fused_")
    for f in os.listdir(src_root):
        shutil.copy(os.path.join(src_root, f), os.path.join(dst, f))
    info = json.load(open(os.path.join(dst, "act_info.json")))
    scale = np.float64(np.exp(-C_SHIFT))
    for s in info["act_func_sets"]:
        if "exp" not in s["act"]:
            continue
        prof = json.load(open(os.path.join(dst, s["profile_json"])))
        order = sorted(prof["func_to_bkt_start_idx"].items(), key=lambda kv: kv[1])
        idx = [i for i, (k, _) in enumerate(order) if k == "exp"][0]
        lo = order[idx][1]
        hi = order[idx + 1][1] if idx + 1 < len(order) else prof["bkt_entry_cnt"]
        path = os.path.join(dst, s["bkt_bin"])
        bkt = np.fromfile(path, dtype=np.float32).reshape(-1, 8).copy()
        for b in range(lo, hi):
            d0, d1, _, _, x0 = bkt[b, :5]
            if x0 <= _ZERO_BELOW:
                bkt[b, 0:4] = 0.0  # masked region: exp -> exact 0
                continue
            if not (d0 > 0 and np.isfinite(d0) and abs(d1 - d0) <= 1e-3 * d0):
                continue  # saturation buckets (inf / 0)
            if x0 > 0:
                # positive side: exp(x - C)
                g = np.float32(np.exp(np.float64(x0) - C_SHIFT))
                bkt[b, 0] = g
                bkt[b, 1] = g
            else:
                # negative side: exp(alpha*x - C) (nearly flat; linear spline)
                g = np.float32(np.exp(_FUSED_ALPHA * np.float64(x0) - C_SHIFT))
                bkt[b, 0] = g
                bkt[b, 1] = np.float32(_FUSED_ALPHA * g)
            bkt[b, 2] = np.float32(0.0)  # cubic terms fault the engine
            bkt[b, 3] = np.float32(0.0)
        bkt.tofile(path)
    return os.path.join(dst, "act_info.json")


def build_nc():
    global _CACHED
    if _CACHED is not None:
        return _CACHED
    os.environ["BASS_ACT_ROOT_JSON_PATH"] = _make_fused_act_root()
    nc = bacc.Bacc("TRN2", target_bir_lowering=False, debug=False,
                   enable_asserts=False, num_devices=NCORES)
    adjT8 = nc.dram_tensor("adjT8", [N, ROWS], FP8, kind="ExternalInput").ap()
    whi = nc.dram_tensor("whi", [N, D], FP8, kind="ExternalInput").ap()
    wlo = nc.dram_tensor("wlo", [N, D], FP8, kind="ExternalInput").ap()
    xT = nc.dram_tensor("xT", [D, N], BF16, kind="ExternalInput").ap()
    a8t = nc.dram_tensor("a8t", [P, 8], BF16, kind="ExternalInput").ap()
    out = nc.dram_tensor("out", [ROWS, D], BF16, kind="ExternalOutput").ap()
    w1scr = nc.dram_tensor("w1scr", [1, ROWS], BF16, kind="Internal").ap()

    from contextlib import ExitStack
    with tile.TileContext(nc) as tc:
        with ExitStack() as ctx:
            _build_kernel(nc, tc, adjT8, whi, wlo, xT, a8t, out, w1scr, ctx)
    nc.compile()
    _CACHED = nc
    return nc


def make_in_maps(input, adj_matrix, a):
    E4 = ml_dtypes.float8_e4m3
    BF = ml_dtypes.bfloat16
    x16 = np.asarray(input, dtype=np.float32).astype(BF)
    x16f = x16.astype(np.float32)
    w_hi = np.ascontiguousarray(x16f.astype(E4))
    w_lo = np.ascontiguousarray((x16f - w_hi.astype(np.float32)).astype(E4))
    adj = np.asarray(adj_matrix)
    a_f = np.asarray(a, dtype=np.float32).reshape(-1)
    a8t = np.ascontiguousarray(a_f.reshape(8, P).T.astype(BF))  # [128, 8]
    xT_full = np.ascontiguousarray(x16.T)                        # [D, N] bf16
    in_maps = []
    for c in range(NCORES):
        rows = slice(c * ROWS, (c + 1) * ROWS)
        # per-core j-rotation: tile 0 is always this core's own rows
        rot = np.roll(np.arange(N), -c * ROWS)
        adjT_c = adj[rows, :].T[rot]          # [N(j rotated), ROWS(i local)]
        adjT8_c = np.ascontiguousarray(
            ((adjT_c.astype(np.float32) - 1.0) * (-MASK_NEG)).astype(E4))
        in_maps.append({
            "adjT8": adjT8_c,
            "whi": np.ascontiguousarray(w_hi[rot]),
            "wlo": np.ascontiguousarray(w_lo[rot]),
            "xT": np.ascontiguousarray(xT_full[:, rot]),
            "a8t": a8t,
        })
    return in_maps


def kernel(input, adj_matrix, a, _trace=False, _tmpdir=None):
    nc = build_nc()
    in_maps = make_in_maps(input, adj_matrix, a)
    try:
        res = run_bass_kernel_spmd(nc, in_maps, core_ids=list(range(NCORES)),
                                   trace=_trace, tmpdir=_tmpdir)
    except ModuleNotFoundError:
        res = run_bass_kernel_spmd(nc, in_maps, core_ids=list(range(NCORES)))
    out = np.concatenate(
        [res.results[c]["out"].astype(np.float32) for c in range(NCORES)],
        axis=0)
    kernel._last_results = res
    return out


# revision 24
# speedup vs baseline: 1.0655x; 1.0655x over previous
"""GAT-style attentive layer on 8 TRN2 NeuronCores — fp8 DoubleRow version.

Math (per reference):
    Wh  = input                      [N, D]   (N=8192, D=512)
    Wh1 = Wh @ a[:D]  (s_i)          [N, 1]
    Wh2 = Wh @ a[D:]  (t_j)          [N, 1]
    e   = leaky_relu(Wh1 + Wh2.T, 0.01)
    e   = where(adj > 0, e, -9e15)
    att = softmax(e, axis=1)
    out = att @ Wh                   [N, D]

Sharding: row-shard the N x N attention across 8 cores (1024 rows each).
Scores are produced transposed, pT[j, i] (j on partitions), so the final
matmul uses pT tiles as the stationary operand.

Key speed tricks (all validated against the TimelineSim cost model):
 - The big matmul runs in fp8e4 (e4m3) with MatmulPerfMode.DoubleRow: one
   PE instruction contracts TWO 128-deep j-tiles at 0.5 cycles per output
   column (4x the bf16 rate per MAC).  Accuracy is held by splitting Wh
   into hi+lo e4m3 planes (w = w_hi + w_lo reconstructs bf16-level w), so
   only the attention weights p carry fp8 noise; measured end-to-end
   rel-err 1.66e-2 against the f32 oracle (gate: 2e-2).
 - p8 values are produced by a single Act pass per score tile: the
   compiler's Exp table is patched to compute exp(lrelu(x) - 1.25), with
   x <= -20 mapped to exact 0.  The -1.25 shift keeps p below e4m3's 240
   max (softmax cancels the shift), lrelu is fused into the table, and
   the adjacency mask is additive ({0, -112} fp8), folded in by the
   score-add.
 - The score-add s_t = (adjT8 + t_j) + bcast_wh1 is a single
   scalar_tensor_tensor op (per-partition t_j rides the scalar slot),
   split between the DVE and gpsimd engines to keep both under the Act
   engine's throughput.  Folding t_j here (instead of the Act bias) lets
   the Act pass batch 4 j-tiles per instruction.
 - Row sums ride a third DoubleRow matmul per pair against a ones pair
   (~1 cycle each); normalization is a reciprocal-multiply on the DVE.

Host-side prep (data marshaling only): dtype casts + transpose/slicing;
all compute (projections, scores, exp, matmul, normalize) runs on device.
"""

import os

import numpy as np
import ml_dtypes

import concourse.bass as bass
import concourse.mybir as mybir
import concourse.tile as tile
from concourse import bacc
from concourse.bass_utils import run_bass_kernel_spmd

N = 8192          # nodes
D = 512           # feature dim
NCORES = 8
ROWS = N // NCORES  # 1024 output rows per core
P = 128
NJT = N // P      # 64 j-tiles
NPAIR = NJT // 2  # 32 j-tile pairs (one DoubleRow contraction each)
IC_W = 512        # i-chunk width (PSUM-limited)
NIC = ROWS // IC_W  # 2 i-chunks
ITPC = IC_W // P  # 4 i-subtiles per chunk

MASK_NEG = -112.0  # additive mask (e4m3-exact); table maps x <= -20 to 0
C_SHIFT = 1.25     # table computes exp(lrelu(x) - C_SHIFT): keeps p < 240

AF = mybir.ActivationFunctionType
ALU = mybir.AluOpType
PM = mybir.MatmulPerfMode
dt = mybir.dt
F32 = dt.float32
BF16 = dt.bfloat16
FP8 = dt.float8e4

# All score-adds run on the DVE: the per-partition t_j scalar rides the
# TensorScalarPtr opcode, which the Pool engine does not implement.
def _add_on_pool(jt: int, ic: int) -> bool:
    return False


def _build_kernel(nc: bass.Bass, tc: tile.TileContext,
                  adjT8: bass.AP, whi: bass.AP, wlo: bass.AP,
                  xT: bass.AP, a8t_d: bass.AP, out: bass.AP,
                  w1scr: bass.AP, ctx):
    pool_const = ctx.enter_context(tc.tile_pool(name="const", bufs=1))
    pool_w = ctx.enter_context(tc.tile_pool(name="w", bufs=1))
    pool_adj = ctx.enter_context(tc.tile_pool(name="adj", bufs=8))
    pool_st = ctx.enter_context(tc.tile_pool(name="st", bufs=3))
    pool_p8 = ctx.enter_context(tc.tile_pool(name="p8", bufs=3))
    pool_outs = ctx.enter_context(tc.tile_pool(name="outs", bufs=2))
    pool_small = ctx.enter_context(tc.tile_pool(name="small", bufs=1))
    pool_psum = ctx.enter_context(tc.tile_pool(name="psum", bufs=1, space="PSUM"))

    # ---- constants / warmup ------------------------------------------------
    warm = pool_const.tile([1, 2], F32, tag="warm", name="warm")
    nc.vector.memset(warm, 0.0)
    nc.scalar.activation(warm, warm, AF.Exp)  # pull ACT_TABLE_LOAD to t~0

    ones2 = pool_const.tile([P, 2, 2], FP8, tag="ones2", name="ones2")
    nc.vector.memset(ones2, 1.0)

    a8t = pool_const.tile([P, 8], BF16, tag="a8t", name="a8t")
    nc.sync.dma_start(a8t, a8t_d)

    # ---- PSUM arena (bank 8): t-columns [*,0:64], row-sums [*,64:80] -------
    # (psum_out takes 7 banks, pass-B 1 more would be 9: the small stuff
    # shares one bank, memset once, all matmuls start=False.)
    arena = pool_psum.tile([P, 512], F32, tag="arena", name="arena")
    nc.vector.memset(arena, 0.0)

    # ---- xT (transposed x, bf16) for wh1 + wh2 projections -----------------
    # Streamed in 8 j-range chunks of [128, 4, 1024] (1MB) through a 3-buf
    # ring, so t-projections for early j-tiles unblock fast and the full
    # 8MB never sits in SBUF.  The host rotates j per core so chunk 0 is
    # always this core's own rows (which also serve wh1).
    pool_xT = ctx.enter_context(tc.tile_pool(name="xT", bufs=2))
    xTc = {}

    def dma_xT_chunk(g):
        t = pool_xT.tile([P, 4, ROWS], BF16, tag="xTc", name="xTc", bufs=2)
        nc.sync.dma_start(
            t, xT[:, bass.ds(g * ROWS, ROWS)].rearrange("(t p) j -> p t j", p=P))
        xTc[g] = t

    dma_xT_chunk(0)

    bcast16 = pool_const.tile([P, ROWS], BF16, tag="bw", name="bw")
    wh1_row = pool_const.tile([1, ROWS], BF16, tag="w1r", name="w1r")
    wh1c_sb = pool_const.tile([P, 8], BF16, tag="w1c", name="w1c")

    def wh1_compute():
        # column-layout a1-projections of the local rows into arena[*,80:88]
        # (same machinery as t_compute), then a tiny SBUF->SBUF transpose DMA
        # to the [1, 1024] row the partition-broadcast wants.  No temporal
        # arena sharing, so nothing serializes behind re-zeros.
        for k in range(8):
            for t in range(4):
                nc.tensor.matmul(arena[:, 80 + k:81 + k],
                                 lhsT=xTc[0][:, t, bass.ds(k * P, P)],
                                 rhs=a8t[:, t:t + 1],
                                 start=False, stop=(t == 3),
                                 skip_group_check=True)
        nc.scalar.copy(wh1c_sb, arena[:, 80:88])
        nc.vector.memset(arena[:, 80:88], 0.0)  # pass-B reuses these columns
        # transpose [128, 8] -> [1, 1024] via a DRAM hop (DRAM APs are
        # free-form; a cross-partition SBUF->SBUF AP is not expressible)
        nc.sync.dma_start(
            w1scr.rearrange("x (t p) -> p (x t)", p=P), wh1c_sb)
        nc.sync.dma_start(wh1_row, w1scr)
        nc.gpsimd.partition_broadcast(bcast16, wh1_row[0:1, :])

    # t_j for ALL j (wh2), in bias-column layout [128, 64]: per j-tile a
    # [128, 1] arena column from 4 k-matmuls (ap_size=1: nearly free on PE).
    wh2_sb = pool_const.tile([P, NJT], F32, tag="wh2", name="wh2")

    def t_compute(g):
        # group g covers j-tiles 8g..8g+7 (chunk g's 1024 columns)
        for k in range(8):
            jt = 8 * g + k
            for t in range(4):
                nc.tensor.matmul(arena[:, jt:jt + 1],
                                 lhsT=xTc[g][:, t, bass.ds(k * P, P)],
                                 rhs=a8t[:, 4 + t:5 + t],
                                 start=False, stop=(t == 3),
                                 skip_group_check=True)
        nc.scalar.copy(wh2_sb[:, bass.ds(8 * g, 8)], arena[:, bass.ds(8 * g, 8)])
        del xTc[g]

    adj_pre = {}

    def dma_adj_pair(pair):
        t = pool_adj.tile([P, 2, ROWS], FP8, tag="adjq", name="adjq", bufs=10)
        nc.sync.dma_start(
            t, adjT8[bass.ds(pair * 2 * P, 2 * P), :]
            .rearrange("(q p) i -> p q i", p=P))
        return t

    adj_pre[0] = dma_adj_pair(0)
    wh1_compute()
    t_compute(0)
    adj_pre[1] = dma_adj_pair(1)
    dma_xT_chunk(1)

    # ---- W planes (resident), interleaved with adj pairs + xT chunks in
    # strict consumption order (the DMA queue is FIFO; order here IS the
    # delivery schedule).
    whq_hi, whq_lo = [], []
    for m in range(16):
        if 2 * m not in adj_pre:
            adj_pre[2 * m] = dma_adj_pair(2 * m)
        thi = pool_w.tile([P, 4, D], FP8, tag=f"whi{m}", name=f"whi{m}")
        nc.sync.dma_start(thi, whi[bass.ds(m * 4 * P, 4 * P), :]
                          .rearrange("(q p) d -> p q d", p=P))
        whq_hi.append(thi)
        if 2 * m + 1 not in adj_pre:
            adj_pre[2 * m + 1] = dma_adj_pair(2 * m + 1)
        tlo = pool_w.tile([P, 4, D], FP8, tag=f"wlo{m}", name=f"wlo{m}")
        nc.sync.dma_start(tlo, wlo[bass.ds(m * 4 * P, 4 * P), :]
                          .rearrange("(q p) d -> p q d", p=P))
        whq_lo.append(tlo)
        if m <= 5:
            dma_xT_chunk(m + 2)
        if 1 <= m <= 7:
            t_compute(m)

    # ---- single pass over j: i-subtiles 0..6 stream through 7 PSUM banks;
    # p8 tiles stay resident so i-subtile 7 runs as a short PE-only pass B.
    NA = 7  # pass-A i-subtiles
    psum_out = [pool_psum.tile([P, D], F32, tag=f"po{i}", name=f"po{i}")
                for i in range(NA)]
    p8_tiles = []

    for quad in range(16):  # 4 j-tiles per quad
        s_q = pool_st.tile([P, 4, ROWS], BF16, tag="s_q", name="s_q", bufs=2)
        for k in range(4):
            jt = 4 * quad + k
            pair = jt // 2
            if pair in adj_pre:
                adjq = adj_pre.pop(pair)
            nc.vector.scalar_tensor_tensor(
                out=s_q[:, k, :], in0=adjq[:, jt % 2, :],
                scalar=wh2_sb[:, jt:jt + 1], in1=bcast16,
                op0=ALU.add, op1=ALU.add)
        p8 = pool_p8.tile([P, 4, ROWS], FP8, tag=f"p8q{quad}",
                          name=f"p8q{quad}", bufs=1)
        p8_tiles.append(p8)
        nc.scalar.activation(p8, s_q, AF.Exp)

        for pp in range(2):
            pair = 2 * quad + pp
            first = pair == 0
            last = pair == NPAIR - 1
            # pass-B (i-subtile 7) interleaved into pass A: row-sum plus
            # the first 432 d-columns accumulate in spare arena columns
            # using the PE's slack; only an 80-wide sweep remains at the
            # tail.
            lhs7 = p8[:, bass.ds(2 * pp, 2), bass.ds(NA * P, P)]
            last = pair == NPAIR - 1
            nc.tensor.matmul(arena[:, 78:80], lhsT=lhs7, rhs=ones2,
                             start=False, stop=last,
                             perf_mode=PM.DoubleRow, skip_group_check=True)
            m2, s2 = pair // 2, bass.ds(2 * (pair % 2), 2)
            nc.tensor.matmul(arena[:, 80:336], lhsT=lhs7,
                             rhs=whq_hi[m2][:, s2, bass.ds(0, 256)],
                             start=False, stop=False,
                             perf_mode=PM.DoubleRow, skip_group_check=True)
            nc.tensor.matmul(arena[:, 80:336], lhsT=lhs7,
                             rhs=whq_lo[m2][:, s2, bass.ds(0, 256)],
                             start=False, stop=last,
                             perf_mode=PM.DoubleRow, skip_group_check=True)
            nc.tensor.matmul(arena[:, 336:512], lhsT=lhs7,
                             rhs=whq_hi[m2][:, s2, bass.ds(256, 176)],
                             start=False, stop=False,
                             perf_mode=PM.DoubleRow, skip_group_check=True)
            nc.tensor.matmul(arena[:, 336:512], lhsT=lhs7,
                             rhs=whq_lo[m2][:, s2, bass.ds(256, 176)],
                             start=False, stop=last,
                             perf_mode=PM.DoubleRow, skip_group_check=True)
            for i4 in range(NA):
                lhs = p8[:, bass.ds(2 * pp, 2), bass.ds(i4 * P, P)]
                nc.tensor.matmul(psum_out[i4], lhsT=lhs,
                                 rhs=whq_hi[pair // 2][:, bass.ds(2 * (pair % 2), 2), :],
                                 start=first, stop=False,
                                 perf_mode=PM.DoubleRow,
                                 skip_group_check=not first)
                nc.tensor.matmul(psum_out[i4], lhsT=lhs,
                                 rhs=whq_lo[pair // 2][:, bass.ds(2 * (pair % 2), 2), :],
                                 start=False, stop=last,
                                 perf_mode=PM.DoubleRow,
                                 skip_group_check=True)
                nc.tensor.matmul(arena[:, 64 + 2 * i4:64 + 2 * i4 + 2],
                                 lhsT=lhs, rhs=ones2,
                                 start=False, stop=last,
                                 perf_mode=PM.DoubleRow,
                                 skip_group_check=True)

    # ---- tail: reciprocals first (dependency-ready order on the DVE),
    # normalize muls split across DVE/Act, sweep-3 (last 80 d-cols of
    # i-subtile 7) into arena[0:80] once the t-columns and row-sums die.
    recip16 = pool_small.tile([P, 16], F32, tag="recip", name="recip")
    nc.vector.reciprocal(recip16[:, 14:16], arena[:, 78:80])
    nc.vector.reciprocal(recip16[:, 0:14], arena[:, 64:78])

    outq = pool_outs.tile([P, NA, D], BF16, tag="outq", name="outq")
    outb = pool_outs.tile([P, D], BF16, tag="outb", name="outb")
    nc.vector.tensor_scalar_mul(outb[:, 0:256], arena[:, 80:336],
                                recip16[:, 14:15])
    nc.scalar.mul(outb[:, 256:432], arena[:, 336:512], recip16[:, 14:15])
    nc.vector.memset(arena[:, 0:80], 0.0)
    for pair in range(NPAIR):
        p8 = p8_tiles[pair // 2]
        lhs7 = p8[:, bass.ds(2 * (pair % 2), 2), bass.ds(NA * P, P)]
        m2, s2 = pair // 2, bass.ds(2 * (pair % 2), 2)
        nc.tensor.matmul(arena[:, 0:80], lhsT=lhs7,
                         rhs=whq_hi[m2][:, s2, bass.ds(432, 80)],
                         start=False, stop=False,
                         perf_mode=PM.DoubleRow, skip_group_check=True)
        nc.tensor.matmul(arena[:, 0:80], lhsT=lhs7,
                         rhs=whq_lo[m2][:, s2, bass.ds(432, 80)],
                         start=False, stop=(pair == NPAIR - 1),
                         perf_mode=PM.DoubleRow, skip_group_check=True)
    for i4 in range(NA):
        recip = recip16[:, 2 * i4:2 * i4 + 1]
        if i4 % 2 == 0:
            nc.vector.tensor_scalar_mul(outq[:, i4, :], psum_out[i4], recip)
        else:
            nc.scalar.mul(outq[:, i4, :], psum_out[i4], recip)
        if i4 % 2 == 1:
            nc.sync.dma_start(
                out[bass.ds((i4 - 1) * P, 2 * P), :].rearrange(
                    "(q p) d -> p q d", p=P), outq[:, i4 - 1:i4 + 1, :])
    nc.scalar.mul(outb[:, 432:512], arena[:, 0:80], recip16[:, 14:15])
    nc.sync.dma_start(out[bass.ds(6 * P, P), :], outq[:, 6, :])
    nc.sync.dma_start(out[bass.ds(NA * P, P), :], outb)


_CACHED = None

_FUSED_ALPHA = 0.01
_ZERO_BELOW = -20.0  # table inputs below this produce exact 0


def _make_fused_act_root() -> str:
    """Copy the compiler's activation-table dir, patching Exp to compute
      x < -20:       exactly 0 (additively-masked scores exp to zero)
      x in [-20, 0): exp(_FUSED_ALPHA*x - C_SHIFT)   (lrelu fused)
      x >= 0:        exp(x - C_SHIFT)
    The -C_SHIFT keeps outputs under e4m3's 240 max; softmax cancels it.
    Returns path to the patched act_info.json."""
    import json
    import shutil
    import tempfile

    from neuronxcc.driver.Job import Job
    from neuronxcc.driver.jobs.support.FindActInfo import findActInfoFile

    src_root = os.path.dirname(findActInfoFile(Job.getPackageDir(), "gen3"))
    dst = tempfile.mkdtemp(prefix="act_root_fused_")
    for f in os.listdir(src_root):
        shutil.copy(os.path.join(src_root, f), os.path.join(dst, f))
    info = json.load(open(os.path.join(dst, "act_info.json")))
    scale = np.float64(np.exp(-C_SHIFT))
    for s in info["act_func_sets"]:
        if "exp" not in s["act"]:
            continue
        prof = json.load(open(os.path.join(dst, s["profile_json"])))
        order = sorted(prof["func_to_bkt_start_idx"].items(), key=lambda kv: kv[1])
        idx = [i for i, (k, _) in enumerate(order) if k == "exp"][0]
        lo = order[idx][1]
        hi = order[idx + 1][1] if idx + 1 < len(order) else prof["bkt_entry_cnt"]
        path = os.path.join(dst, s["bkt_bin"])
        bkt = np.fromfile(path, dtype=np.float32).reshape(-1, 8).copy()
        for b in range(lo, hi):
            d0, d1, _, _, x0 = bkt[b, :5]
            if x0 <= _ZERO_BELOW:
                bkt[b, 0:4] = 0.0  # masked region: exp -> exact 0
                continue
            if not (d0 > 0 and np.isfinite(d0) and abs(d1 - d0) <= 1e-3 * d0):
                continue  # saturation buckets (inf / 0)
            if x0 > 0:
                # positive side: exp(x - C)
                g = np.float32(np.exp(np.float64(x0) - C_SHIFT))
                bkt[b, 0] = g
                bkt[b, 1] = g
            else:
                # negative side: exp(alpha*x - C) (nearly flat; linear spline)
                g = np.float32(np.exp(_FUSED_ALPHA * np.float64(x0) - C_SHIFT))
                bkt[b, 0] = g
                bkt[b, 1] = np.float32(_FUSED_ALPHA * g)
            bkt[b, 2] = np.float32(0.0)  # cubic terms fault the engine
            bkt[b, 3] = np.float32(0.0)
        bkt.tofile(path)
    return os.path.join(dst, "act_info.json")


def build_nc():
    global _CACHED
    if _CACHED is not None:
        return _CACHED
    os.environ["BASS_ACT_ROOT_JSON_PATH"] = _make_fused_act_root()
    nc = bacc.Bacc("TRN2", target_bir_lowering=False, debug=False,
                   enable_asserts=False, num_devices=NCORES)
    adjT8 = nc.dram_tensor("adjT8", [N, ROWS], FP8, kind="ExternalInput").ap()
    whi = nc.dram_tensor("whi", [N, D], FP8, kind="ExternalInput").ap()
    wlo = nc.dram_tensor("wlo", [N, D], FP8, kind="ExternalInput").ap()
    xT = nc.dram_tensor("xT", [D, N], BF16, kind="ExternalInput").ap()
    a8t = nc.dram_tensor("a8t", [P, 8], BF16, kind="ExternalInput").ap()
    out = nc.dram_tensor("out", [ROWS, D], BF16, kind="ExternalOutput").ap()
    w1scr = nc.dram_tensor("w1scr", [1, ROWS], BF16, kind="Internal").ap()

    from contextlib import ExitStack
    with tile.TileContext(nc) as tc:
        with ExitStack() as ctx:
            _build_kernel(nc, tc, adjT8, whi, wlo, xT, a8t, out, w1scr, ctx)
    nc.compile()
    _CACHED = nc
    return nc


def make_in_maps(input, adj_matrix, a):
    E4 = ml_dtypes.float8_e4m3
    BF = ml_dtypes.bfloat16
    x16 = np.asarray(input, dtype=np.float32).astype(BF)
    x16f = x16.astype(np.float32)
    w_hi = np.ascontiguousarray(x16f.astype(E4))
    w_lo = np.ascontiguousarray((x16f - w_hi.astype(np.float32)).astype(E4))
    adj = np.asarray(adj_matrix)
    a_f = np.asarray(a, dtype=np.float32).reshape(-1)
    a8t = np.ascontiguousarray(a_f.reshape(8, P).T.astype(BF))  # [128, 8]
    xT_full = np.ascontiguousarray(x16.T)                        # [D, N] bf16
    in_maps = []
    for c in range(NCORES):
        rows = slice(c * ROWS, (c + 1) * ROWS)
        # per-core j-rotation: tile 0 is always this core's own rows
        rot = np.roll(np.arange(N), -c * ROWS)
        adjT_c = adj[rows, :].T[rot]          # [N(j rotated), ROWS(i local)]
        adjT8_c = np.ascontiguousarray(
            ((adjT_c.astype(np.float32) - 1.0) * (-MASK_NEG)).astype(E4))
        in_maps.append({
            "adjT8": adjT8_c,
            "whi": np.ascontiguousarray(w_hi[rot]),
            "wlo": np.ascontiguousarray(w_lo[rot]),
            "xT": np.ascontiguousarray(xT_full[:, rot]),
            "a8t": a8t,
        })
    return in_maps


def kernel(input, adj_matrix, a, _trace=False, _tmpdir=None):
    nc = build_nc()
    in_maps = make_in_maps(input, adj_matrix, a)
    try:
        res = run_bass_kernel_spmd(nc, in_maps, core_ids=list(range(NCORES)),
                                   trace=_trace, tmpdir=_tmpdir)
    except ModuleNotFoundError:
        res = run_bass_kernel_spmd(nc, in_maps, core_ids=list(range(NCORES)))
    out = np.concatenate(
        [res.results[c]["out"].astype(np.float32) for c in range(NCORES)],
        axis=0)
    kernel._last_results = res
    return out
